# revision 1
# baseline (speedup 1.0000x reference)
"""BCH/RS systematic encoder kernel for Trainium2 (8 NeuronCores, data parallel).

Computes out = concat([msg, (msg @ Gp) mod 2], axis=-1) for
msg [16384, 1000] f32 of 0/1 bits and Gp [1000, 256] f32 of 0/1 bits.

Design (per core, 2048 rows, 16 chunks of 128):
  - SWDGE cast-load msg chunk f32 -> bf16 SBUF (0/1 exact in bf16)
  - SWDGE cast-store bf16 -> f32 to out[:, :1000] (systematic copy-through)
  - DMA xbar transpose (2-byte) 128x128 blocks: msg natural -> msgT [k, m]
  - 8 accumulating bf16 matmuls: psum[m,256] += msgT_k.T @ Gp_k (fp32 accum, exact)
  - DVE tensor_scalar mod 2.0 on psum -> SBUF f32
  - store parity to out[:, 1000:1256]
HBM traffic/core = 8.19 MB read + 10.29 MB write (the minimum).
"""

import os
import sys

import numpy as np

if os.path.isdir("/opt/trn_rl_repo") and "/opt/trn_rl_repo" not in sys.path:
    sys.path.insert(0, "/opt/trn_rl_repo")

import ml_dtypes

import concourse.bacc as bacc
import concourse.mybir as mybir
import concourse.tile as tile
from concourse.bass_utils import run_bass_kernel_spmd

BATCH = 16384
MSG = 1000
NPAR = 256
NCORES = 8
ROWS = BATCH // NCORES  # 2048
P = 128
KCH = 8  # k chunks; padded K = 1024
KPAD = KCH * P

# test.py pokes these for profiling
TRACE = False
LAST_RESULT = None

_CACHE = {}


def build_nc(rows=ROWS):
    """Emit the Bass/Tile IR for one core handling `rows` rows."""
    mch = rows // P
    nc = bacc.Bacc("TRN2", target_bir_lowering=False, debug=False)
    msg = nc.dram_tensor("msg", [rows, MSG], mybir.dt.float32, kind="ExternalInput")
    gp = nc.dram_tensor("gp", [P, KCH * NPAR], mybir.dt.bfloat16, kind="ExternalInput")
    out = nc.dram_tensor(
        "out", [rows, MSG + NPAR], mybir.dt.float32, kind="ExternalOutput"
    )

    SC = 2  # m-chunks per superchunk (SWDGE/DVE batching granularity)
    n_super = mch // SC
    LAG = 1  # stores trail compute by this many superchunks
    msg3 = msg[:, :].rearrange("(s c p) k -> s c p k", c=SC, p=P)
    out3 = out[:, :].rearrange("(s c p) k -> s c p k", c=SC, p=P)

    with tile.TileContext(nc) as tc:
        with (
            tc.tile_pool(name="gpool", bufs=1) as gpool,
            # every superchunk's a-tile is resident at once: loads all run
            # upfront and never wait on a recycled slot (or a store's SWDGE
            # semaphore lane)
            tc.tile_pool(name="apool", bufs=n_super + 1) as apool,
            tc.tile_pool(name="bpool", bufs=6) as bpool,
            tc.tile_pool(name="cpool", bufs=4) as cpool,
            tc.tile_pool(name="epool", bufs=4) as epool,
            tc.tile_pool(name="ppool", bufs=8, space="PSUM") as ppool,
        ):
            # Gp resident in SBUF: gsb[q, kb*256 + n] = Gp_padded[kb*128 + q, n]
            gsb = gpool.tile([P, KCH * NPAR], mybir.dt.bfloat16)
            nc.sync.dma_start(out=gsb[:, :], in_=gp[:, :])

            a_tiles = {}
            es = {}

            # row stride must keep every a[:, c, :] slice 32B-aligned for the
            # xbar transpose: 1264 bf16 = 2528 B = 79*32
            ROWP = 1264

            def emit_load(si):
                # full output row in bf16: cols 0:1000 msg, 1000:1256 parity.
                # No zero-pad memset: the last k-chunk matmul contracts K=104,
                # so the PE never reads the transposed garbage rows.
                a = apool.tile([P, SC, ROWP], mybir.dt.bfloat16, tag="a")
                nc.gpsimd.dma_start(
                    out=a[:, :, 0:MSG], in_=msg3[si, :, :, :].rearrange("c p k -> p c k")
                )
                a_tiles[si] = a

            def emit_compute(si):
                a = a_tiles[si]
                # per-chunk xbar transpose: b[q, c*KCH + kb, p] = a[p, c, kb*128+q]
                # all on ONE HWDGE ring: concurrent xbar transposes from two
                # rings corrupt each other (shared xbar; this Tile does not
                # cross-engine-serialize them)
                b = bpool.tile([P, SC * KCH, P], mybir.dt.bfloat16, tag="b")
                for c in range(SC):
                    nc.sync.dma_start(
                        out=b[:, c * KCH : (c + 1) * KCH, :],
                        in_=a[:, c, 0:KPAD],
                        transpose=True,
                    )
                # both chunks accumulate side by side in one PSUM bank
                acc = ppool.tile([P, SC * NPAR], mybir.dt.float32, tag="acc")
                for c in range(SC):
                    for kb in range(KCH):
                        kk = P if kb < KCH - 1 else MSG - (KCH - 1) * P  # 104 tail
                        nc.tensor.matmul(
                            acc[:, c * NPAR : (c + 1) * NPAR],
                            b[0:kk, c * KCH + kb, :],
                            gsb[0:kk, kb * NPAR : (kb + 1) * NPAR],
                            start=(kb == 0),
                            stop=(kb == KCH - 1),
                        )
                # exact-integer f32 -> i32 eviction in ONE op on idle ACT
                c_i32 = cpool.tile([P, SC, NPAR], mybir.dt.int32, tag="c")
                nc.scalar.copy(
                    c_i32[:, :, :].rearrange("p c n -> p (c n)"), acc[:, :]
                )
                # mod 2 == AND 1 (bitVec op cannot cast, keep i32)
                e = epool.tile([P, SC, NPAR], mybir.dt.int32, tag="e")
                nc.vector.tensor_scalar(
                    e[:, :, :], c_i32[:, :, :], 1, None, mybir.AluOpType.bitwise_and
                )
                # parity into the output-row tile (0/1 exact in bf16)
                nc.vector.tensor_copy(a[:, :, MSG : MSG + NPAR], e[:, :, :])

            def emit_store(si):
                # single cast-store of the full rows: [p, c, 1256] bf16 -> f32
                a = a_tiles.pop(si)
                nc.gpsimd.dma_start(
                    out=out3[si, :, :, :].rearrange("c p k -> p c k"),
                    in_=a[:, :, 0 : MSG + NPAR],
                )

            for it in range(n_super):
                emit_load(it)
            # zero the transpose pad columns once per (fresh) slot, batched on
            # DVE before the compute chain (keeps CoreSim's uninit checker
            # happy; PE never reads those rows thanks to the K=104 tail)
            for it in range(n_super):
                nc.vector.memset(a_tiles[it][:, :, MSG:KPAD], 0)
            for it in range(n_super + LAG):
                if it < n_super:
                    emit_compute(it)
                k = it - LAG
                if 0 <= k < n_super:
                    emit_store(k)

    nc.compile()
    return nc


def prep_gp(Gp):
    """Pad Gp to 1024 rows and swizzle to the [128, 8*256] bf16 SBUF layout."""
    gp = np.asarray(Gp, dtype=np.float32)
    gp_pad = np.zeros((KPAD, NPAR), dtype=np.float32)
    gp_pad[:MSG] = gp
    gsw = gp_pad.reshape(KCH, P, NPAR).transpose(1, 0, 2).reshape(P, KCH * NPAR)
    return np.ascontiguousarray(gsw).astype(ml_dtypes.bfloat16)


def kernel(message_bits, Gp):
    global LAST_RESULT
    msg = np.ascontiguousarray(np.asarray(message_bits, dtype=np.float32))
    assert msg.shape == (BATCH, MSG), msg.shape
    gsw = prep_gp(Gp)

    if "nc" not in _CACHE:
        _CACHE["nc"] = build_nc()
    nc = _CACHE["nc"]

    in_maps = [
        {"msg": msg[i * ROWS : (i + 1) * ROWS], "gp": gsw} for i in range(NCORES)
    ]
    res = run_bass_kernel_spmd(
        nc, in_maps, core_ids=list(range(NCORES)), trace=TRACE
    )
    LAST_RESULT = res
    return np.concatenate([r["out"] for r in res.results], axis=0)



# revision 3
# speedup vs baseline: 1.3244x; 1.3244x over previous
"""BCH/RS systematic encoder kernel for Trainium2 (8 NeuronCores, data parallel).

Computes out = concat([msg, (msg @ Gp) mod 2], axis=-1) for
msg [16384, 1000] f32 of 0/1 bits and Gp [1000, 256] f32 of 0/1 bits.

Design v2 (per core, 2048 rows, 8 superchunks of 2x128):
  - HWDGE plain f32 load of msg chunk straight into the f32 output-row tile
    (copy-through region), HWDGE plain f32 store of finished rows: both HBM
    transfers ride hardware-DGE rings at full rate, no SWDGE descriptor
    generation, no cast-DMA.
  - ACT casts msg f32 -> fp8e4 (0/1 exact). The fp8 tile viewed as u16 pairs
    is xbar-transposed (HALF the bytes of a bf16 transpose): each u16 holds
    msg[m, 2f] and msg[m, 2f+1], so transposed partition q carries k = 2q
    and 2q+1 interleaved along m.
  - PE DoubleRow fp8 matmuls consume exactly that pair layout: one
    instruction contracts 256 k (2 slots of 128 partitions), 4 instructions
    per 128-row chunk, accumulated f32 in PSUM (exact integer sums).
  - ACT evicts PSUM f32 -> i32, DVE ANDs with 1 (mod 2), DVE copies i32 ->
    f32 parity straight into the output-row tile.
HBM traffic/core = 8.19 MB read + 10.29 MB write (the minimum) + 2.1 MB
SBUF->SBUF transpose on the same SDMA engines.
"""

import os
import sys

import numpy as np

if os.path.isdir("/opt/trn_rl_repo") and "/opt/trn_rl_repo" not in sys.path:
    sys.path.insert(0, "/opt/trn_rl_repo")

import ml_dtypes

import concourse.bacc as bacc
import concourse.mybir as mybir
import concourse.tile as tile
from concourse.bass_utils import run_bass_kernel_spmd

BATCH = 16384
MSG = 1000
NPAR = 256
NCORES = 8
ROWS = BATCH // NCORES  # 2048
P = 128
KB = 4  # k pair-blocks of 256; padded K = 1024
KPAD = KB * 2 * P
SC = 2  # m-chunks per superchunk

# test.py pokes these for profiling
TRACE = False
LAST_RESULT = None

_CACHE = {}

F8 = mybir.dt.float8e4
U16 = mybir.dt.uint16


def build_nc(rows=ROWS):
    """Emit the Bass/Tile IR for one core handling `rows` rows."""
    mch = rows // P
    n_super = mch // SC
    nc = bacc.Bacc("TRN2", target_bir_lowering=False, debug=False)
    msg = nc.dram_tensor("msg", [rows, MSG], mybir.dt.float32, kind="ExternalInput")
    gp = nc.dram_tensor("gp", [P, KB, 2, NPAR], F8, kind="ExternalInput")
    out = nc.dram_tensor(
        "out", [rows, MSG + NPAR], mybir.dt.float32, kind="ExternalOutput"
    )

    msg3 = msg[:, :].rearrange("(s c p) k -> s c p k", c=SC, p=P)
    out3 = out[:, :].rearrange("(s c p) k -> s c p k", c=SC, p=P)

    with tile.TileContext(nc) as tc:
        with (
            tc.tile_pool(name="gpool", bufs=1) as gpool,
            tc.tile_pool(name="opool", bufs=4) as opool,
            tc.tile_pool(name="fpool", bufs=3) as fpool,
            tc.tile_pool(name="tpool", bufs=3) as tpool,
            tc.tile_pool(name="cpool", bufs=3) as cpool,
            tc.tile_pool(name="epool", bufs=3) as epool,
            tc.tile_pool(name="ppool", bufs=4, space="PSUM") as ppool,
        ):
            # Gp resident in SBUF in DoubleRow layout:
            # gsb[q, b, j, n] = Gp_padded[256*b + 2*q + j, n]
            gsb = gpool.tile([P, KB, 2, NPAR], F8)
            nc.sync.dma_start(out=gsb[:, :, :, :], in_=gp[:, :, :, :])

            for si in range(n_super):
                # full f32 output rows; cols 0:1000 filled by the load itself
                o = opool.tile([P, SC, MSG + NPAR], mybir.dt.float32, tag="o")
                nc.sync.dma_start(
                    out=o[:, :, 0:MSG],
                    in_=msg3[si, :, :, :].rearrange("c p k -> p c k"),
                )
                # fp8 copy for the PE (0/1 exact); pad columns zeroed so the
                # padded k-range contributes nothing
                f8 = fpool.tile([P, SC, KPAD], F8, tag="f8")
                nc.scalar.copy(f8[:, :, 0:MSG], o[:, :, 0:MSG])
                nc.vector.memset(f8[:, :, MSG:KPAD], 0)
                # u16-pair-view xbar transpose (all on ONE HWDGE ring):
                # t[q, c, b, m] (u16) = fp8 pair (msg[m, 256b+2q], msg[m, 256b+2q+1])
                t = tpool.tile([P, SC, KB, P], U16, tag="t")
                for c in range(SC):
                    nc.sync.dma_start(
                        out=t[:, c, :, :],
                        in_=f8[:, c, :].bitcast(U16),
                        transpose=True,
                    )
                # plain fp8 matmuls: per pair-block b, the even/odd fp8 slot
                # of each transposed u16 is a stride-2 [128, 128] weights AP
                # contracting k = 256b + 2q + j against the matching
                # host-swizzled Gp rows
                acc = ppool.tile([P, SC * NPAR], mybir.dt.float32, tag="acc")
                for c in range(SC):
                    for b in range(KB):
                        lhsT2 = (
                            t[:, c, b, :]
                            .bitcast(F8)
                            .rearrange("q (m j) -> q j m", j=2)
                        )
                        for j in range(2):
                            nc.tensor.matmul(
                                acc[:, c * NPAR : (c + 1) * NPAR],
                                lhsT2[:, j, :],
                                gsb[:, b, j, :],
                                start=(b == 0 and j == 0),
                                stop=(b == KB - 1 and j == 1),
                            )
                # exact-integer f32 -> i32 eviction on ACT
                ci = cpool.tile([P, SC, NPAR], mybir.dt.int32, tag="ci")
                nc.scalar.copy(ci[:, :, :].rearrange("p c n -> p (c n)"), acc[:, :])
                # mod 2 == AND 1 (bitVec op cannot cast, keep i32)
                e = epool.tile([P, SC, NPAR], mybir.dt.int32, tag="e")
                nc.vector.tensor_scalar(
                    e[:, :, :], ci[:, :, :], 1, None, mybir.AluOpType.bitwise_and
                )
                # parity i32 -> f32 straight into the output-row tile
                nc.vector.tensor_copy(o[:, :, MSG : MSG + NPAR], e[:, :, :])
                # plain f32 store of the finished rows on the other HWDGE ring
                nc.scalar.dma_start(
                    out=out3[si, :, :, :].rearrange("c p k -> p c k"),
                    in_=o[:, :, :],
                )

    nc.compile()
    return nc


def prep_gp(Gp):
    """Pad Gp to 1024 rows and swizzle to DoubleRow [128, 4, 2, 256] fp8.

    gsw[q, b, j, n] = Gp_pad[256*b + 2*q + j, n]
    """
    gp = np.asarray(Gp, dtype=np.float32)
    gp_pad = np.zeros((KPAD, NPAR), dtype=np.float32)
    gp_pad[:MSG] = gp
    gsw = gp_pad.reshape(KB, P, 2, NPAR).transpose(1, 0, 2, 3)
    return np.ascontiguousarray(gsw).astype(ml_dtypes.float8_e4m3)


def kernel(message_bits, Gp):
    global LAST_RESULT
    msg = np.ascontiguousarray(np.asarray(message_bits, dtype=np.float32))
    assert msg.shape == (BATCH, MSG), msg.shape
    gsw = prep_gp(Gp)

    if "nc" not in _CACHE:
        _CACHE["nc"] = build_nc()
    nc = _CACHE["nc"]

    in_maps = [
        {"msg": msg[i * ROWS : (i + 1) * ROWS], "gp": gsw} for i in range(NCORES)
    ]
    res = run_bass_kernel_spmd(
        nc, in_maps, core_ids=list(range(NCORES)), trace=TRACE
    )
    LAST_RESULT = res
    return np.concatenate([r["out"] for r in res.results], axis=0)
